# revision 19
# baseline (speedup 1.0000x reference)
"""GQA attention (llama3-style RoPE, causal) on 8 trn2 NeuronCores.

Sharding: tensor-parallel over KV-head groups for q/k/v, column-parallel for
the output projection. Core i owns kv-head i and its 4 query heads:
wq[:, i*512:(i+1)*512], wk/wv[:, i*128:(i+1)*128]. After attention, each
core's [4 heads x 128, T] output block (bf16) is AllGathered so every core
holds attT = [4096, T]; core i then computes out[:, i*512:(i+1)*512] =
att @ wo[:, i*512:(i+1)*512] locally. The host concatenates column shards.

All matmul operands are bf16 (fp32 streams at 2 cycles/row on trn2, bf16 at
1 and gets fast-weight-load); PSUM accumulation stays fp32. RoPE runs in
fp32 off PSUM. Softmax: pT = exp(sT/sqrt(d)) with paired [128,1024] psum
tiles so one ACTIVATE covers two k-blocks; row-sum l via ones-matmul into
partition 0 of a bank that is then overwritten by the K=1 broadcast matmul
of 1/l = exp(-ln(l)) (both activations share one ACT table set, keeping the
slow DVE reciprocal off the critical path and gpsimd collective-only).
Normalization of block (g,h) is deferred into the next block's emission so
the ln/exp/broadcast latency hides under the next scores pair. o_proj of
group g is emitted late enough (attn0,1,2,op0,attn3,op1,op2,op3) that every
AllGather + readback completes before the PE reaches its consumer. v is
transposed via DMA xbar (dma_start_transpose), not the PE array.
"""

import numpy as np

H, KV, HD, HID = 32, 8, 128, 4096
T = 2048
N_CORES = 8
QH = H // KV            # 4 query heads per core
DQ = QH * HD            # 512
KT = HID // 128         # 32 contraction tiles for projections
KC = 8                  # k-chunks per weight/x sub-tile DMA
G = 4                   # tq groups of 512
GW = T // G             # 512

THETA, FACTOR, HI_FF, LO_FF, ORIG_MAX = 500000.0, 8.0, 4.0, 1.0, 8192

_CACHE = {}


def _rope_tables():
    inv = 1.0 / (THETA ** (np.arange(0, HD, 2, dtype=np.float64) / HD))
    wavelen = 2.0 * np.pi / inv
    low_wl = ORIG_MAX / LO_FF
    high_wl = ORIG_MAX / HI_FF
    smooth = (ORIG_MAX / wavelen - LO_FF) / (HI_FF - LO_FF)
    scaled = np.where(wavelen > low_wl, inv / FACTOR, inv)
    mid = (wavelen <= low_wl) & (wavelen >= high_wl)
    scaled = np.where(mid, (1 - smooth) * inv / FACTOR + smooth * inv, scaled)
    inv32 = scaled.astype(np.float32)
    pos = np.arange(T, dtype=np.float32)
    freqs = pos[:, None] * inv32[None, :]          # [T, 64]
    emb = np.concatenate([freqs, freqs], axis=-1)  # [T, 128]
    cosT = np.ascontiguousarray(np.cos(emb).T)     # [128, T]
    sinT = np.ascontiguousarray(np.sin(emb).T)
    return cosT, sinT


def _causal_pair_masks():
    # pt pair tile is [s=128, 2*512]: halves are k-blocks j=2v, 2v+1 of the
    # diagonal group vs the tq window [g*512, (g+1)*512). keep where tq >= s.
    import ml_dtypes
    tri = np.triu(np.ones((128, 128), dtype=np.float32))
    masks = np.zeros((2, 128, 2 * GW), dtype=np.float32)
    for v in range(2):
        for half in range(2):
            j = 2 * v + half          # diag-group block index 0..3
            for c in range(4):        # tq 128-blocks within the window
                blk = masks[v][:, half * GW + c * 128:half * GW + (c + 1) * 128]
                if c > j:
                    blk[:] = 1.0
                elif c == j:
                    blk[:] = tri
    return masks.astype(ml_dtypes.bfloat16)


def _build_program():
    import concourse.bacc as bacc
    import concourse.mybir as mybir
    from concourse.tile import TileContext

    f32 = mybir.dt.float32
    f32r = mybir.dt.float32r
    bf16 = mybir.dt.bfloat16
    EXPF = mybir.ActivationFunctionType.Exp

    nc = bacc.Bacc("TRN2", target_bir_lowering=False, debug=False,
                   num_devices=N_CORES)

    xTd = nc.dram_tensor("xT", [HID, T], bf16, kind="ExternalInput")
    wqd = nc.dram_tensor("wq", [HID, DQ], bf16, kind="ExternalInput")
    wkd = nc.dram_tensor("wk", [HID, HD], bf16, kind="ExternalInput")
    wvd = nc.dram_tensor("wv", [HID, HD], bf16, kind="ExternalInput")
    wod = nc.dram_tensor("wo", [HID, DQ], bf16, kind="ExternalInput")
    cosd = nc.dram_tensor("cosT", [HD, T], bf16, kind="ExternalInput")
    sind = nc.dram_tensor("sinT", [HD, T], bf16, kind="ExternalInput")
    maskd = nc.dram_tensor("masks", [2, HD, 2 * GW], bf16, kind="ExternalInput")
    onesd = nc.dram_tensor("ones", [128, 1], bf16, kind="ExternalInput")
    seld = nc.dram_tensor("sel", [128, QH * 128], f32, kind="ExternalInput")
    outd = nc.dram_tensor("out", [T, DQ], f32, kind="ExternalOutput")

    # collective staging: per-g attention-out shard and gathered attT;
    # group 3 ships in two head-pair halves so its AllGather starts two
    # blocks earlier and the o_proj tail shrinks
    oT_dram = [nc.dram_tensor(f"oT{g}", [DQ, GW], bf16) for g in range(G - 1)]
    attT = [nc.dram_tensor(f"attT{g}", [H * HD, GW], bf16, addr_space="Shared")
            for g in range(G - 1)]
    oT3_dram = [nc.dram_tensor(f"oT3{s}", [2 * HD, GW], bf16) for s in "ab"]
    attT3 = [nc.dram_tensor(f"attT3{s}", [N_CORES * 2 * HD, GW], bf16,
                            addr_space="Shared") for s in "ab"]
    warm_in = nc.dram_tensor("warm_in", [1, 64], bf16)
    warm_out = nc.dram_tensor("warm_out", [N_CORES, 64], bf16,
                              addr_space="Shared")

    def r(ap):
        return ap.bitcast(f32r)

    def split3(ap, k, p=128):
        # DRAM [k*p, c] -> [p, k, c] (partition-first iteration order)
        return ap.rearrange("(k p) c -> p k c", p=p)

    scale = float(1.0 / np.sqrt(HD))
    NSUB = KT // KC  # weight/x sub-tiles per group

    with TileContext(nc) as tc:
        with tc.tile_pool(name="persist", bufs=1) as pers:
            qT = [pers.tile([128, T], bf16, tag=f"qT{h}", name=f"qT{h}")
                  for h in range(QH)]
            kT = pers.tile([128, T], bf16, tag="kT")
            vsb = pers.tile([128, T], bf16, tag="vsb")   # v [s,d] tiles
            ones = pers.tile([128, 1], bf16, tag="ones")
            sel = pers.tile([128, QH * 128], f32r, tag="sel")
            pm = pers.tile([128, 2 * 2 * GW], bf16, tag="pm")
            nc.sync.dma_start(ones[:], onesd[:])
            nc.sync.dma_start(sel[:], r(seld[:]))

            # absorb first-collective setup cost while projections run
            nc.gpsimd.collective_compute(
                "AllGather", mybir.AluOpType.bypass,
                replica_groups=[list(range(N_CORES))],
                ins=[warm_in[:]], outs=[warm_out[:]],
            )

            # ---------------- phase B: projections + RoPE ----------------
            with (
                tc.tile_pool(name="wts", bufs=1) as wtp,
                tc.tile_pool(name="cs", bufs=1) as csp,
                tc.tile_pool(name="xg", bufs=2) as xgp,
                tc.tile_pool(name="rtmp", bufs=2) as rtp,
                tc.tile_pool(name="vtmp", bufs=2) as vtp,
                tc.tile_pool(name="ppsA", bufs=2, space="PSUM") as ppsA,
                tc.tile_pool(name="ppsB", bufs=1, space="PSUM") as ppsB,
            ):
                # wq split per head so head h's k-loop only waits on its
                # own ~1 MB of DMA; x split in NSUB chunks per group
                wqs = [wtp.tile([128, KT * 128], bf16, tag=f"wqh{h}",
                                name=f"wqh{h}") for h in range(QH)]
                wks = wtp.tile([128, KT * HD], bf16, tag="wks")
                wvs = wtp.tile([128, KT * HD], bf16, tag="wvs")
                cos = csp.tile([128, T], bf16, tag="cos")
                sin = csp.tile([128, T], bf16, tag="sin")

                def wq_ap(k, h):
                    return wqs[h][:, k * 128:(k + 1) * 128]

                xg_tiles = {}

                def load_xg(g):
                    gs = slice(g * GW, (g + 1) * GW)
                    tiles = [xgp.tile([128, KC * GW], bf16, tag=f"xg{c}",
                                      name=f"xg{g}_{c}") for c in range(NSUB)]
                    for c in range(NSUB):
                        nc.sync.dma_start(
                            tiles[c][:].rearrange("p (k c) -> p k c", k=KC),
                            xTd[c * KC * 128:(c + 1) * KC * 128, gs]
                            .rearrange("(k p) c -> p k c", p=128))
                    xg_tiles[g] = tiles

                def xg_ap(g, k):
                    c, kk = divmod(k, KC)
                    return xg_tiles[g][c][:, kk * GW:(kk + 1) * GW]

                def load_wqh(h):
                    nc.sync.dma_start(
                        wqs[h][:].rearrange("p (k c) -> p k c", k=KT),
                        wqd[:, h * 128:(h + 1) * 128]
                        .rearrange("(k p) c -> p k c", p=128))

                # startup DMA order: first-needed first
                load_wqh(0)
                load_xg(0)
                load_wqh(1)
                load_wqh(2)
                load_wqh(3)
                nc.sync.dma_start(cos[:], cosd[:])
                nc.sync.dma_start(sin[:], sind[:])
                nc.sync.dma_start(
                    pm[:].rearrange("p (m c) -> p m c", m=2),
                    maskd[:].rearrange("m p c -> p m c"))
                nc.sync.dma_start(
                    wks[:].rearrange("p (k c) -> p k c", k=KT), split3(wkd[:], KT))
                nc.sync.dma_start(
                    wvs[:].rearrange("p (k c) -> p k c", k=KT), split3(wvd[:], KT))

                def rope(src_ps, dst_ap, gs):
                    # dst = src*cos + rotate_half(src)*sin   (src is PSUM f32)
                    t1 = rtp.tile([128, GW], f32, tag="t1")
                    nc.vector.tensor_mul(t1[:], src_ps[:], cos[:, gs])
                    rot = rtp.tile([128, GW], f32, tag="rot")
                    nc.scalar.mul(rot[0:64, :], src_ps[64:128, :], -1.0)
                    nc.scalar.copy(rot[64:128, :], src_ps[0:64, :])
                    rot2 = rtp.tile([128, GW], f32, tag="rot2")
                    nc.vector.tensor_mul(rot2[:], rot[:], sin[:, gs])
                    nc.vector.tensor_add(dst_ap, t1[:], rot2[:])

                for g in range(G):
                    gs = slice(g * GW, (g + 1) * GW)
                    if g + 1 < G:
                        load_xg(g + 1)
                    # q heads: h-outer k-inner; qp0/qp1 double-buffered so the
                    # next g's first matmuls never wait on this g's rope drain
                    for h in range(QH):
                        pool = ppsA if h < 2 else ppsB
                        qps = pool.tile([128, GW], f32, tag=f"qp{h}",
                                        name=f"qp{h}_{g}")
                        for k in range(KT):
                            nc.tensor.matmul(qps[:], wq_ap(k, h), xg_ap(g, k),
                                             start=(k == 0), stop=(k == KT - 1))
                        rope(qps, qT[h][:, gs], gs)
                    kps = ppsB.tile([128, GW], f32, tag="kp", name=f"kp{g}")
                    for k in range(KT):
                        nc.tensor.matmul(kps[:], wks[:, k * HD:(k + 1) * HD],
                                         xg_ap(g, k),
                                         start=(k == 0), stop=(k == KT - 1))
                    rope(kps, kT[:, gs], gs)
                    vps = ppsB.tile([128, GW], f32, tag="vp", name=f"vp{g}")
                    for k in range(KT):
                        nc.tensor.matmul(vps[:], wvs[:, k * HD:(k + 1) * HD],
                                         xg_ap(g, k),
                                         start=(k == 0), stop=(k == KT - 1))
                    vT = vtp.tile([128, GW], bf16, tag="vT")
                    nc.vector.tensor_copy(vT[:], vps[:])
                    for ts in range(4):
                        nc.sync.dma_start_transpose(
                            vsb[:, (4 * g + ts) * 128:(4 * g + ts + 1) * 128],
                            vT[:, ts * 128:(ts + 1) * 128])

            # -------- attention + AllGather + column-sharded o_proj --------
            with (
                tc.tile_pool(name="wo", bufs=1) as wop,
                tc.tile_pool(name="pt", bufs=3) as ptp,
                tc.tile_pool(name="oT", bufs=2) as oTp,
                tc.tile_pool(name="attg", bufs=2) as agp,
                tc.tile_pool(name="attg3", bufs=1) as ag3p,
                tc.tile_pool(name="ob", bufs=2) as obp,
                tc.tile_pool(name="nrm", bufs=2) as nrm,
                # "sp" slots serve both the paired score tiles and (as
                # [:, 0:512] views) the o_proj accumulators
                tc.tile_pool(name="sps", bufs=2, space="PSUM") as aps,
                tc.tile_pool(name="lbl", bufs=2, space="PSUM") as lbp,
                tc.tile_pool(name="ops", bufs=2, space="PSUM") as opp,
            ):
                wos = wop.tile([128, KT * DQ], bf16, tag="wos")
                nc.sync.dma_start(
                    wos[:].rearrange("p (k c) -> p k c", k=KT), split3(wod[:], KT))

                oTg_tiles = {}
                l4_tiles = {}
                pending = []  # deferred per-group normalize + ship closure

                def flush_pending():
                    while pending:
                        pending.pop(0)()

                def attn_block(g, h, flush_at=None):
                    gs = slice(g * GW, (g + 1) * GW)
                    npair = 2 * g + 2
                    lbl = lbp.tile([128, GW], f32, tag="lbl", name=f"lbl{g}_{h}")
                    ops = opp.tile([128, GW], f32, tag="op", name=f"op{g}_{h}")
                    if h == 0:
                        oTg_tiles[g] = oTp.tile([128, QH * GW], bf16, tag="oTg",
                                                name=f"oTg{g}")
                        l4_tiles[g] = nrm.tile([128, GW], f32, tag="l4",
                                               name=f"l4_{g}")
                        nc.vector.memset(l4_tiles[g][:], 1.0)
                    for p in range(npair):
                        j0 = 2 * p
                        diag = p >= 2 * g
                        # diagonal k-blocks only cover tq >= block start:
                        # trim scores/l/av to the live column range
                        offs = [(2 * (p - 2 * g) + half) * 128 if diag else 0
                                for half in range(2)]
                        sp = aps.tile([128, 2 * GW], f32, tag="sp")
                        for half in range(2):
                            off = offs[half]
                            nc.tensor.matmul(
                                sp[:, half * GW + off:(half + 1) * GW],
                                kT[:, (j0 + half) * 128:(j0 + half + 1) * 128],
                                qT[h][:, g * GW + off:(g + 1) * GW],
                                start=True, stop=True)
                        if p == flush_at:
                            # previous normalize + AllGather hide here,
                            # under this block's early scores pairs
                            flush_pending()
                        pt = ptp.tile([128, 2 * GW], bf16, tag="pt")
                        if not diag:
                            nc.scalar.activation(pt[:], sp[:], EXPF, scale=scale)
                        else:
                            v = p - 2 * g
                            for half in range(2):
                                off = offs[half]
                                hw = half * GW
                                nc.scalar.activation(
                                    pt[:, hw + off:hw + GW],
                                    sp[:, hw + off:hw + GW], EXPF, scale=scale)
                                # only the 128-wide diagonal tile needs the
                                # triangular mask; later columns are all-keep
                                nc.vector.tensor_mul(
                                    pt[:, hw + off:hw + off + 128],
                                    pt[:, hw + off:hw + off + 128],
                                    pm[:, v * 2 * GW + hw + off:
                                        v * 2 * GW + hw + off + 128])
                        for half in range(2):
                            st = (p == 0 and half == 0)
                            sp_ = (p == npair - 1 and half == 1)
                            off = offs[half]
                            h0 = half * GW
                            nc.tensor.matmul(lbl[0:1, off:GW], ones[:],
                                             pt[:, h0 + off:h0 + GW],
                                             start=st, stop=sp_)
                            nc.tensor.matmul(
                                ops[:, off:GW],
                                vsb[:, (j0 + half) * 128:(j0 + half + 1) * 128],
                                pt[:, h0 + off:h0 + GW], start=st, stop=sp_)
                    # stash l and the unnormalized head output; the divide
                    # happens in the deferred per-group batch
                    nc.vector.tensor_copy(
                        l4_tiles[g][h * 32:h * 32 + 1, :], lbl[0:1, :])
                    nc.vector.tensor_copy(
                        oTg_tiles[g][:, h * GW:(h + 1) * GW], ops[:])

                def normalize_heads(g, hs):
                    # divide heads `hs` of group g by their l (batched recip)
                    oTg = oTg_tiles[g]
                    l4i = nrm.tile([128, GW], f32, tag="l4i",
                                   name=f"l4i{g}_{hs[0]}")
                    nc.vector.reciprocal(l4i[:], l4_tiles[g][:])
                    l4r = nrm.tile([128, GW], f32r, tag="l4r",
                                   name=f"l4r{g}_{hs[0]}")
                    nc.vector.tensor_copy(l4r[:], l4i[:])
                    bc = opp.tile([128, GW], f32, tag="op", name=f"bc{g}_{hs[0]}")
                    for h in hs:
                        nc.tensor.matmul(bc[:], sel[:, h * 128:(h + 1) * 128],
                                         l4r[:], start=True, stop=True)
                        nc.vector.tensor_mul(
                            oTg[:, h * GW:(h + 1) * GW],
                            oTg[:, h * GW:(h + 1) * GW], bc[:])

                def ship3_half(s):
                    # AllGather heads (2s, 2s+1) of group 3
                    normalize_heads(3, (2 * s, 2 * s + 1))
                    nc.scalar.dma_start(
                        oT3_dram[s][:].rearrange("(h p) c -> p h c", p=128),
                        oTg_tiles[3][:, s * 2 * GW:(s + 1) * 2 * GW]
                        .rearrange("p (h c) -> p h c", h=2))
                    nc.gpsimd.collective_compute(
                        "AllGather", mybir.AluOpType.bypass,
                        replica_groups=[list(range(N_CORES))],
                        ins=[oT3_dram[s][:]], outs=[attT3[s][:]],
                    )
                    attg[f"3{s}"] = ag3p.tile([128, 16 * GW], bf16,
                                              tag=f"ag3{s}", name=f"ag3{s}")
                    nc.sync.dma_start(
                        attg[f"3{s}"][:].rearrange("p (k c) -> p k c", k=16),
                        split3(attT3[s][:], 16))

                def normalize_and_ship(g):
                    normalize_heads(g, range(QH))
                    oTg = oTg_tiles[g]
                    nc.scalar.dma_start(
                        oT_dram[g][:].rearrange("(h p) c -> p h c", p=128),
                        oTg[:].rearrange("p (h c) -> p h c", h=QH))
                    nc.gpsimd.collective_compute(
                        "AllGather", mybir.AluOpType.bypass,
                        replica_groups=[list(range(N_CORES))],
                        ins=[oT_dram[g][:]], outs=[attT[g][:]],
                    )
                    attg[g] = agp.tile([128, KT * GW], bf16, tag="attg",
                                       name=f"attg{g}")
                    nc.sync.dma_start(
                        attg[g][:].rearrange("p (k c) -> p k c", k=KT),
                        split3(attT[g][:], KT))

                attg = {}

                def oproj_tile(g, t):
                    # out rows [(4g+t)*128, +128), all 512 owned columns;
                    # accumulator borrows an "sp" slot (left half)
                    op = aps.tile([128, 2 * GW], f32, tag="sp",
                                  name=f"oj{g}_{t}")
                    if g < G - 1:
                        chunks = [(attg[g], k, k) for k in range(KT)]
                    else:
                        # half s holds rank-major head pairs (2s, 2s+1):
                        # its chunk c is global head 4*(c//2) + 2*s + c%2
                        chunks = [(attg[f"3{s}"], c, 4 * (c // 2) + 2 * s + c % 2)
                                  for s in range(2) for c in range(16)]
                    for k, (src_t, c, w) in enumerate(chunks):
                        nc.tensor.matmul(
                            op[:, 0:DQ],
                            src_t[:, c * GW + t * 128:c * GW + (t + 1) * 128],
                            wos[:, w * DQ:(w + 1) * DQ],
                            start=(k == 0), stop=(k == KT - 1))
                    ob = obp.tile([128, DQ], f32, tag="ob")
                    nc.vector.tensor_copy(ob[:], op[:, 0:DQ])
                    nc.scalar.dma_start(
                        outd[(4 * g + t) * 128:(4 * g + t + 1) * 128, :], ob[:])

                def attn_group(g):
                    for h in range(QH):
                        attn_block(g, h, flush_at=3 if h == 0 else None)
                    pending.append(lambda: normalize_and_ship(g))

                def attn_group3():
                    attn_block(3, 0, flush_at=3)
                    attn_block(3, 1)
                    pending.append(lambda: ship3_half(0))
                    attn_block(3, 2, flush_at=3)
                    attn_block(3, 3)
                    ship3_half(1)

                attn_group(0)
                attn_group(1)
                attn_group(2)
                for t in range(4):
                    oproj_tile(0, t)
                attn_group3()
                for t in range(4):
                    oproj_tile(1, t)
                for t in range(4):
                    oproj_tile(2, t)
                for t in range(4):
                    oproj_tile(3, t)

    nc.compile()
    return nc


def _get_program():
    if "nc" not in _CACHE:
        _CACHE["nc"] = _build_program()
    return _CACHE["nc"]


def kernel(x, wq, wk, wv, wo):
    import ml_dtypes
    from concourse.bass_utils import run_bass_kernel_spmd

    nc = _get_program()
    bf16 = ml_dtypes.bfloat16

    x2 = np.asarray(x, dtype=np.float32).reshape(T, HID)
    xT = np.ascontiguousarray(x2.T).astype(bf16)
    cosT, sinT = _rope_tables()
    cosT = cosT.astype(bf16)
    sinT = sinT.astype(bf16)
    masks = _causal_pair_masks()
    ones = np.ones((128, 1), dtype=bf16)
    sel = np.zeros((128, QH * 128), dtype=np.float32)
    for h in range(QH):
        sel[h * 32, h * 128:(h + 1) * 128] = 1.0
    warm = np.zeros((1, 64), dtype=bf16)

    wq = np.asarray(wq, dtype=np.float32)
    wk = np.asarray(wk, dtype=np.float32)
    wv = np.asarray(wv, dtype=np.float32)
    wo = np.asarray(wo, dtype=np.float32)

    in_maps = []
    for i in range(N_CORES):
        in_maps.append({
            "xT": xT,
            "wq": np.ascontiguousarray(wq[:, i * DQ:(i + 1) * DQ]).astype(bf16),
            "wk": np.ascontiguousarray(wk[:, i * HD:(i + 1) * HD]).astype(bf16),
            "wv": np.ascontiguousarray(wv[:, i * HD:(i + 1) * HD]).astype(bf16),
            "wo": np.ascontiguousarray(wo[:, i * DQ:(i + 1) * DQ]).astype(bf16),
            "cosT": cosT,
            "sinT": sinT,
            "masks": masks,
            "ones": ones,
            "sel": sel,
            "warm_in": warm,
        })

    _CACHE["last_in_maps"] = in_maps
    res = run_bass_kernel_spmd(nc, in_maps, list(range(N_CORES)))
    _CACHE["last_result"] = res
    out = np.empty((T, HID), dtype=np.float32)
    for i in range(N_CORES):
        out[:, i * DQ:(i + 1) * DQ] = res.results[i]["out"]
    return out.reshape(1, T, HID)


# revision 20
# speedup vs baseline: 1.1572x; 1.1572x over previous
"""GQA attention (llama3-style RoPE, causal) on 8 trn2 NeuronCores.

Sharding: tensor-parallel over KV-head groups for q/k/v, column-parallel for
the output projection. Core i owns kv-head i and its 4 query heads:
wq[:, i*512:(i+1)*512], wk/wv[:, i*128:(i+1)*128]. After attention, each
core's [4 heads x 128, T] output block (bf16) is AllGathered so every core
holds attT = [4096, T]; core i then computes out[:, i*512:(i+1)*512] =
att @ wo[:, i*512:(i+1)*512] locally. The host concatenates column shards.

All matmul operands are bf16 (fp32 streams at 2 cycles/row on trn2, bf16 at
1 and gets fast-weight-load); PSUM accumulation stays fp32. RoPE runs in
fp32 off PSUM. Softmax: pT = exp(sT/sqrt(d)) with paired [128,1024] psum
tiles so one ACTIVATE covers two k-blocks; row-sum l via ones-matmul into
partition 0 of a bank that is then overwritten by the K=1 broadcast matmul
of 1/l = exp(-ln(l)) (both activations share one ACT table set, keeping the
slow DVE reciprocal off the critical path and gpsimd collective-only).
Normalization of block (g,h) is deferred into the next block's emission so
the ln/exp/broadcast latency hides under the next scores pair. o_proj of
group g is emitted late enough (attn0,1,2,op0,attn3,op1,op2,op3) that every
AllGather + readback completes before the PE reaches its consumer. v is
transposed via DMA xbar (dma_start_transpose), not the PE array.
"""

import numpy as np

H, KV, HD, HID = 32, 8, 128, 4096
T = 2048
N_CORES = 8
QH = H // KV            # 4 query heads per core
DQ = QH * HD            # 512
KT = HID // 128         # 32 contraction tiles for projections
KC = 8                  # k-chunks per weight/x sub-tile DMA
G = 4                   # tq groups of 512
GW = T // G             # 512

THETA, FACTOR, HI_FF, LO_FF, ORIG_MAX = 500000.0, 8.0, 4.0, 1.0, 8192

_CACHE = {}


def _rope_tables():
    inv = 1.0 / (THETA ** (np.arange(0, HD, 2, dtype=np.float64) / HD))
    wavelen = 2.0 * np.pi / inv
    low_wl = ORIG_MAX / LO_FF
    high_wl = ORIG_MAX / HI_FF
    smooth = (ORIG_MAX / wavelen - LO_FF) / (HI_FF - LO_FF)
    scaled = np.where(wavelen > low_wl, inv / FACTOR, inv)
    mid = (wavelen <= low_wl) & (wavelen >= high_wl)
    scaled = np.where(mid, (1 - smooth) * inv / FACTOR + smooth * inv, scaled)
    inv32 = scaled.astype(np.float32)
    pos = np.arange(T, dtype=np.float32)
    freqs = pos[:, None] * inv32[None, :]          # [T, 64]
    emb = np.concatenate([freqs, freqs], axis=-1)  # [T, 128]
    cosT = np.ascontiguousarray(np.cos(emb).T)     # [128, T]
    sinT = np.ascontiguousarray(np.sin(emb).T)
    return cosT, sinT


def _causal_pair_masks():
    # pt pair tile is [s=128, 2*512]: halves are k-blocks j=2v, 2v+1 of the
    # diagonal group vs the tq window [g*512, (g+1)*512). keep where tq >= s.
    import ml_dtypes
    tri = np.triu(np.ones((128, 128), dtype=np.float32))
    masks = np.zeros((2, 128, 2 * GW), dtype=np.float32)
    for v in range(2):
        for half in range(2):
            j = 2 * v + half          # diag-group block index 0..3
            for c in range(4):        # tq 128-blocks within the window
                blk = masks[v][:, half * GW + c * 128:half * GW + (c + 1) * 128]
                if c > j:
                    blk[:] = 1.0
                elif c == j:
                    blk[:] = tri
    return masks.astype(ml_dtypes.bfloat16)


def _build_program():
    import concourse.bacc as bacc
    import concourse.mybir as mybir
    from concourse.tile import TileContext

    f32 = mybir.dt.float32
    f32r = mybir.dt.float32r
    bf16 = mybir.dt.bfloat16
    EXPF = mybir.ActivationFunctionType.Exp

    nc = bacc.Bacc("TRN2", target_bir_lowering=False, debug=False,
                   num_devices=N_CORES)

    xTd = nc.dram_tensor("xT", [HID, T], bf16, kind="ExternalInput")
    wqd = nc.dram_tensor("wq", [HID, DQ], bf16, kind="ExternalInput")
    wkd = nc.dram_tensor("wk", [HID, HD], bf16, kind="ExternalInput")
    wvd = nc.dram_tensor("wv", [HID, HD], bf16, kind="ExternalInput")
    wod = nc.dram_tensor("wo", [HID, DQ], bf16, kind="ExternalInput")
    cosd = nc.dram_tensor("cosT", [HD, T], bf16, kind="ExternalInput")
    sind = nc.dram_tensor("sinT", [HD, T], bf16, kind="ExternalInput")
    maskd = nc.dram_tensor("masks", [2, HD, 2 * GW], bf16, kind="ExternalInput")
    onesd = nc.dram_tensor("ones", [128, 1], bf16, kind="ExternalInput")
    seld = nc.dram_tensor("sel", [128, QH * 128], f32, kind="ExternalInput")
    outd = nc.dram_tensor("out", [T, DQ], f32, kind="ExternalOutput")

    # collective staging: per-g attention-out shard and gathered attT;
    # group 3 ships in two head-pair halves so its AllGather starts two
    # blocks earlier and the o_proj tail shrinks
    oT_dram = [nc.dram_tensor(f"oT{g}", [DQ, GW], bf16) for g in range(G - 1)]
    attT = [nc.dram_tensor(f"attT{g}", [H * HD, GW], bf16, addr_space="Shared")
            for g in range(G - 1)]
    oT3_dram = [nc.dram_tensor(f"oT3{s}", [2 * HD, GW], bf16) for s in "ab"]
    attT3 = [nc.dram_tensor(f"attT3{s}", [N_CORES * 2 * HD, GW], bf16,
                            addr_space="Shared") for s in "ab"]

    def r(ap):
        return ap.bitcast(f32r)

    def split3(ap, k, p=128):
        # DRAM [k*p, c] -> [p, k, c] (partition-first iteration order)
        return ap.rearrange("(k p) c -> p k c", p=p)

    scale = float(1.0 / np.sqrt(HD))
    NSUB = KT // KC  # weight/x sub-tiles per group

    with TileContext(nc) as tc:
        with tc.tile_pool(name="persist", bufs=1) as pers:
            qT = [pers.tile([128, T], bf16, tag=f"qT{h}", name=f"qT{h}")
                  for h in range(QH)]
            kT = pers.tile([128, T], bf16, tag="kT")
            vsb = pers.tile([128, T], bf16, tag="vsb")   # v [s,d] tiles
            ones = pers.tile([128, 1], bf16, tag="ones")
            sel = pers.tile([128, QH * 128], f32r, tag="sel")
            pm = pers.tile([128, 2 * 2 * GW], bf16, tag="pm")
            nc.sync.dma_start(ones[:], onesd[:])
            nc.sync.dma_start(sel[:], r(seld[:]))

            # ---------------- phase B: projections + RoPE ----------------
            with (
                tc.tile_pool(name="wts", bufs=1) as wtp,
                tc.tile_pool(name="cs", bufs=1) as csp,
                tc.tile_pool(name="xg", bufs=2) as xgp,
                tc.tile_pool(name="rtmp", bufs=2) as rtp,
                tc.tile_pool(name="vtmp", bufs=4) as vtp,
                tc.tile_pool(name="ppsA", bufs=2, space="PSUM") as ppsA,
                tc.tile_pool(name="ppsB", bufs=1, space="PSUM") as ppsB,
            ):
                # wq split per head so head h's k-loop only waits on its
                # own ~1 MB of DMA; x split in NSUB chunks per group
                wqs = [wtp.tile([128, KT * 128], bf16, tag=f"wqh{h}",
                                name=f"wqh{h}") for h in range(QH)]
                wks = wtp.tile([128, KT * HD], bf16, tag="wks")
                wvs = wtp.tile([128, KT * HD], bf16, tag="wvs")
                cos = csp.tile([128, T], bf16, tag="cos")
                sin = csp.tile([128, T], bf16, tag="sin")

                def wq_ap(k, h):
                    return wqs[h][:, k * 128:(k + 1) * 128]

                xg_tiles = {}
                vT_tiles = []

                def load_xg(g):
                    gs = slice(g * GW, (g + 1) * GW)
                    tiles = [xgp.tile([128, KC * GW], bf16, tag=f"xg{c}",
                                      name=f"xg{g}_{c}") for c in range(NSUB)]
                    for c in range(NSUB):
                        nc.sync.dma_start(
                            tiles[c][:].rearrange("p (k c) -> p k c", k=KC),
                            xTd[c * KC * 128:(c + 1) * KC * 128, gs]
                            .rearrange("(k p) c -> p k c", p=128))
                    xg_tiles[g] = tiles

                def xg_ap(g, k):
                    c, kk = divmod(k, KC)
                    return xg_tiles[g][c][:, kk * GW:(kk + 1) * GW]

                def load_wqh(h):
                    nc.sync.dma_start(
                        wqs[h][:].rearrange("p (k c) -> p k c", k=KT),
                        wqd[:, h * 128:(h + 1) * 128]
                        .rearrange("(k p) c -> p k c", p=128))

                # startup DMA order: first-needed first
                load_wqh(0)
                load_xg(0)
                load_wqh(1)
                load_wqh(2)
                load_wqh(3)
                nc.sync.dma_start(cos[:], cosd[:])
                nc.sync.dma_start(sin[:], sind[:])
                nc.sync.dma_start(
                    pm[:].rearrange("p (m c) -> p m c", m=2),
                    maskd[:].rearrange("m p c -> p m c"))
                nc.sync.dma_start(
                    wks[:].rearrange("p (k c) -> p k c", k=KT), split3(wkd[:], KT))
                nc.sync.dma_start(
                    wvs[:].rearrange("p (k c) -> p k c", k=KT), split3(wvd[:], KT))

                def rope(src_ps, dst_ap, gs):
                    # dst = src*cos + rotate_half(src)*sin   (src is PSUM f32)
                    t1 = rtp.tile([128, GW], f32, tag="t1")
                    nc.vector.tensor_mul(t1[:], src_ps[:], cos[:, gs])
                    rot = rtp.tile([128, GW], f32, tag="rot")
                    nc.scalar.mul(rot[0:64, :], src_ps[64:128, :], -1.0)
                    nc.scalar.copy(rot[64:128, :], src_ps[0:64, :])
                    rot2 = rtp.tile([128, GW], f32, tag="rot2")
                    nc.vector.tensor_mul(rot2[:], rot[:], sin[:, gs])
                    nc.vector.tensor_add(dst_ap, t1[:], rot2[:])

                for g in range(G):
                    gs = slice(g * GW, (g + 1) * GW)
                    if g + 1 < G:
                        load_xg(g + 1)
                    # q heads: h-outer k-inner; qp0/qp1 double-buffered so the
                    # next g's first matmuls never wait on this g's rope drain
                    for h in range(QH):
                        pool = ppsA if h < 2 else ppsB
                        qps = pool.tile([128, GW], f32, tag=f"qp{h}",
                                        name=f"qp{h}_{g}")
                        for k in range(KT):
                            nc.tensor.matmul(qps[:], wq_ap(k, h), xg_ap(g, k),
                                             start=(k == 0), stop=(k == KT - 1))
                        rope(qps, qT[h][:, gs], gs)
                    kps = ppsB.tile([128, GW], f32, tag="kp", name=f"kp{g}")
                    for k in range(KT):
                        nc.tensor.matmul(kps[:], wks[:, k * HD:(k + 1) * HD],
                                         xg_ap(g, k),
                                         start=(k == 0), stop=(k == KT - 1))
                    rope(kps, kT[:, gs], gs)
                    vps = ppsB.tile([128, GW], f32, tag="vp", name=f"vp{g}")
                    for k in range(KT):
                        nc.tensor.matmul(vps[:], wvs[:, k * HD:(k + 1) * HD],
                                         xg_ap(g, k),
                                         start=(k == 0), stop=(k == KT - 1))
                    vT = vtp.tile([128, GW], bf16, tag="vT",
                                  name=f"vT{g}")
                    nc.vector.tensor_copy(vT[:], vps[:])
                    vT_tiles.append(vT)

                for g in range(G):
                    for ts in range(4):
                        nc.sync.dma_start_transpose(
                            vsb[:, (4 * g + ts) * 128:(4 * g + ts + 1) * 128],
                            vT_tiles[g][:, ts * 128:(ts + 1) * 128])

            # -------- attention + AllGather + column-sharded o_proj --------
            with (
                tc.tile_pool(name="wo", bufs=1) as wop,
                tc.tile_pool(name="pt", bufs=3) as ptp,
                tc.tile_pool(name="oT", bufs=2) as oTp,
                tc.tile_pool(name="attg", bufs=2) as agp,
                tc.tile_pool(name="attg3", bufs=1) as ag3p,
                tc.tile_pool(name="ob", bufs=2) as obp,
                tc.tile_pool(name="nrm", bufs=2) as nrm,
                # "sp" slots serve both the paired score tiles and (as
                # [:, 0:512] views) the o_proj accumulators
                tc.tile_pool(name="sps", bufs=2, space="PSUM") as aps,
                tc.tile_pool(name="lbl", bufs=2, space="PSUM") as lbp,
                tc.tile_pool(name="ops", bufs=2, space="PSUM") as opp,
            ):
                wos = wop.tile([128, KT * DQ], bf16, tag="wos")
                nc.sync.dma_start(
                    wos[:].rearrange("p (k c) -> p k c", k=KT), split3(wod[:], KT))

                oTg_tiles = {}
                l4_tiles = {}
                pending = []  # deferred per-group normalize + ship closure

                def flush_pending():
                    while pending:
                        pending.pop(0)()

                def attn_block(g, h, flush_at=None):
                    gs = slice(g * GW, (g + 1) * GW)
                    npair = 2 * g + 2
                    lbl = lbp.tile([128, GW], f32, tag="lbl", name=f"lbl{g}_{h}")
                    ops = opp.tile([128, GW], f32, tag="op", name=f"op{g}_{h}")
                    if h == 0:
                        oTg_tiles[g] = oTp.tile([128, QH * GW], bf16, tag="oTg",
                                                name=f"oTg{g}")
                        l4_tiles[g] = nrm.tile([128, GW], f32, tag="l4",
                                               name=f"l4_{g}")
                        nc.vector.memset(l4_tiles[g][:], 1.0)
                    for p in range(npair):
                        j0 = 2 * p
                        diag = p >= 2 * g
                        # diagonal k-blocks only cover tq >= block start:
                        # trim scores/l/av to the live column range
                        offs = [(2 * (p - 2 * g) + half) * 128 if diag else 0
                                for half in range(2)]
                        sp = aps.tile([128, 2 * GW], f32, tag="sp")
                        for half in range(2):
                            off = offs[half]
                            nc.tensor.matmul(
                                sp[:, half * GW + off:(half + 1) * GW],
                                kT[:, (j0 + half) * 128:(j0 + half + 1) * 128],
                                qT[h][:, g * GW + off:(g + 1) * GW],
                                start=True, stop=True)
                        if p == flush_at:
                            # previous normalize + AllGather hide here,
                            # under this block's early scores pairs
                            flush_pending()
                        pt = ptp.tile([128, 2 * GW], bf16, tag="pt")
                        if not diag:
                            nc.scalar.activation(pt[:], sp[:], EXPF, scale=scale)
                        else:
                            v = p - 2 * g
                            for half in range(2):
                                off = offs[half]
                                hw = half * GW
                                nc.scalar.activation(
                                    pt[:, hw + off:hw + GW],
                                    sp[:, hw + off:hw + GW], EXPF, scale=scale)
                                # only the 128-wide diagonal tile needs the
                                # triangular mask; later columns are all-keep
                                nc.vector.tensor_mul(
                                    pt[:, hw + off:hw + off + 128],
                                    pt[:, hw + off:hw + off + 128],
                                    pm[:, v * 2 * GW + hw + off:
                                        v * 2 * GW + hw + off + 128])
                        for half in range(2):
                            st = (p == 0 and half == 0)
                            sp_ = (p == npair - 1 and half == 1)
                            off = offs[half]
                            h0 = half * GW
                            nc.tensor.matmul(lbl[0:1, off:GW], ones[:],
                                             pt[:, h0 + off:h0 + GW],
                                             start=st, stop=sp_)
                            nc.tensor.matmul(
                                ops[:, off:GW],
                                vsb[:, (j0 + half) * 128:(j0 + half + 1) * 128],
                                pt[:, h0 + off:h0 + GW], start=st, stop=sp_)
                    # stash l and the unnormalized head output; the divide
                    # happens in the deferred per-group batch
                    nc.vector.tensor_copy(
                        l4_tiles[g][h * 32:h * 32 + 1, :], lbl[0:1, :])
                    nc.vector.tensor_copy(
                        oTg_tiles[g][:, h * GW:(h + 1) * GW], ops[:])

                def normalize_heads(g, hs):
                    # divide heads `hs` of group g by their l (batched recip)
                    oTg = oTg_tiles[g]
                    l4i = nrm.tile([128, GW], f32, tag="l4i",
                                   name=f"l4i{g}_{hs[0]}")
                    nc.vector.reciprocal(l4i[:], l4_tiles[g][:])
                    l4r = nrm.tile([128, GW], f32r, tag="l4r",
                                   name=f"l4r{g}_{hs[0]}")
                    nc.vector.tensor_copy(l4r[:], l4i[:])
                    bc = opp.tile([128, GW], f32, tag="op", name=f"bc{g}_{hs[0]}")
                    for h in hs:
                        nc.tensor.matmul(bc[:], sel[:, h * 128:(h + 1) * 128],
                                         l4r[:], start=True, stop=True)
                        nc.vector.tensor_mul(
                            oTg[:, h * GW:(h + 1) * GW],
                            oTg[:, h * GW:(h + 1) * GW], bc[:])

                def ship3_half(s):
                    # AllGather heads (2s, 2s+1) of group 3
                    normalize_heads(3, (2 * s, 2 * s + 1))
                    nc.scalar.dma_start(
                        oT3_dram[s][:].rearrange("(h p) c -> p h c", p=128),
                        oTg_tiles[3][:, s * 2 * GW:(s + 1) * 2 * GW]
                        .rearrange("p (h c) -> p h c", h=2))
                    nc.gpsimd.collective_compute(
                        "AllGather", mybir.AluOpType.bypass,
                        replica_groups=[list(range(N_CORES))],
                        ins=[oT3_dram[s][:]], outs=[attT3[s][:]],
                    )
                    attg[f"3{s}"] = ag3p.tile([128, 16 * GW], bf16,
                                              tag=f"ag3{s}", name=f"ag3{s}")
                    nc.sync.dma_start(
                        attg[f"3{s}"][:].rearrange("p (k c) -> p k c", k=16),
                        split3(attT3[s][:], 16))

                def normalize_and_ship(g):
                    normalize_heads(g, range(QH))
                    oTg = oTg_tiles[g]
                    nc.scalar.dma_start(
                        oT_dram[g][:].rearrange("(h p) c -> p h c", p=128),
                        oTg[:].rearrange("p (h c) -> p h c", h=QH))
                    nc.gpsimd.collective_compute(
                        "AllGather", mybir.AluOpType.bypass,
                        replica_groups=[list(range(N_CORES))],
                        ins=[oT_dram[g][:]], outs=[attT[g][:]],
                    )
                    attg[g] = agp.tile([128, KT * GW], bf16, tag="attg",
                                       name=f"attg{g}")
                    nc.sync.dma_start(
                        attg[g][:].rearrange("p (k c) -> p k c", k=KT),
                        split3(attT[g][:], KT))

                attg = {}

                def oproj_tile(g, t):
                    # out rows [(4g+t)*128, +128), all 512 owned columns;
                    # accumulator borrows an "sp" slot (left half)
                    op = aps.tile([128, 2 * GW], f32, tag="sp",
                                  name=f"oj{g}_{t}")
                    if g < G - 1:
                        chunks = [(attg[g], k, k) for k in range(KT)]
                    else:
                        # half s holds rank-major head pairs (2s, 2s+1):
                        # its chunk c is global head 4*(c//2) + 2*s + c%2
                        chunks = [(attg[f"3{s}"], c, 4 * (c // 2) + 2 * s + c % 2)
                                  for s in range(2) for c in range(16)]
                    for k, (src_t, c, w) in enumerate(chunks):
                        nc.tensor.matmul(
                            op[:, 0:DQ],
                            src_t[:, c * GW + t * 128:c * GW + (t + 1) * 128],
                            wos[:, w * DQ:(w + 1) * DQ],
                            start=(k == 0), stop=(k == KT - 1))
                    ob = obp.tile([128, DQ], f32, tag="ob")
                    nc.vector.tensor_copy(ob[:], op[:, 0:DQ])
                    nc.scalar.dma_start(
                        outd[(4 * g + t) * 128:(4 * g + t + 1) * 128, :], ob[:])

                def attn_group(g):
                    for h in range(QH):
                        attn_block(g, h, flush_at=3 if h == 0 else None)
                    pending.append(lambda: normalize_and_ship(g))

                def attn_group3():
                    attn_block(3, 0, flush_at=3)
                    attn_block(3, 1)
                    pending.append(lambda: ship3_half(0))
                    attn_block(3, 2, flush_at=3)
                    attn_block(3, 3)
                    ship3_half(1)

                attn_group(0)
                attn_group(1)
                attn_group(2)
                for t in range(4):
                    oproj_tile(0, t)
                attn_group3()
                for t in range(4):
                    oproj_tile(1, t)
                for t in range(4):
                    oproj_tile(2, t)
                for t in range(4):
                    oproj_tile(3, t)

    nc.compile()
    return nc


def _get_program():
    if "nc" not in _CACHE:
        _CACHE["nc"] = _build_program()
    return _CACHE["nc"]


def kernel(x, wq, wk, wv, wo):
    import ml_dtypes
    from concourse.bass_utils import run_bass_kernel_spmd

    nc = _get_program()
    bf16 = ml_dtypes.bfloat16

    x2 = np.asarray(x, dtype=np.float32).reshape(T, HID)
    xT = np.ascontiguousarray(x2.T).astype(bf16)
    cosT, sinT = _rope_tables()
    cosT = cosT.astype(bf16)
    sinT = sinT.astype(bf16)
    masks = _causal_pair_masks()
    ones = np.ones((128, 1), dtype=bf16)
    sel = np.zeros((128, QH * 128), dtype=np.float32)
    for h in range(QH):
        sel[h * 32, h * 128:(h + 1) * 128] = 1.0

    wq = np.asarray(wq, dtype=np.float32)
    wk = np.asarray(wk, dtype=np.float32)
    wv = np.asarray(wv, dtype=np.float32)
    wo = np.asarray(wo, dtype=np.float32)

    in_maps = []
    for i in range(N_CORES):
        in_maps.append({
            "xT": xT,
            "wq": np.ascontiguousarray(wq[:, i * DQ:(i + 1) * DQ]).astype(bf16),
            "wk": np.ascontiguousarray(wk[:, i * HD:(i + 1) * HD]).astype(bf16),
            "wv": np.ascontiguousarray(wv[:, i * HD:(i + 1) * HD]).astype(bf16),
            "wo": np.ascontiguousarray(wo[:, i * DQ:(i + 1) * DQ]).astype(bf16),
            "cosT": cosT,
            "sinT": sinT,
            "masks": masks,
            "ones": ones,
            "sel": sel,
        })

    _CACHE["last_in_maps"] = in_maps
    res = run_bass_kernel_spmd(nc, in_maps, list(range(N_CORES)))
    _CACHE["last_result"] = res
    out = np.empty((T, HID), dtype=np.float32)
    for i in range(N_CORES):
        out[:, i * DQ:(i + 1) * DQ] = res.results[i]["out"]
    return out.reshape(1, T, HID)
